# revision 9
# baseline (speedup 1.0000x reference)
"""Trainium2 Bass kernel for an MoE routing module.

Strategy: data-parallel over the batch — each of the 8 NeuronCores runs the
full pipeline (gating -> top-2 -> expert MLPs) for its 8 samples. All
data-dependent expert selection is done with indirect-DMA gathers driven by
index tiles computed on device; there are no collectives and no registers.

v4 (fp8 DoubleRow, software-pipelined):
  - gating: emb table is bf16; ONE dma_gather(transpose=True) per 4-sample
    group (2048 tokens) lands tokens in [d-partition, s] layout; pooling is
    one DVE free-axis reduce into pts[d, j, sample] (mean folded into
    gate_w1 on host). Gate MLP stays fp32 so top-2 matches the fp32
    reference (margin/noise ratio measured 5.4x for this seed).
  - experts: token embeddings gathered bf16 (pre-scaled x512 on host),
    PE-transposed against identity into fp32 PSUM; the PSUM->SBUF copy
    casts to fp8 e4m3 (split across Vector/Scalar engines). W1 is fp8
    (x512) in one merged per-expert row with the bf16 W2hi/W2lo/b1/b2
    region read through a bitcast view (one indirect gather per expert
    use). The big [S,D]@[D,H] matmul runs DoubleRow fp8 (2 k-subtiles per
    pass, contiguous columns). relu+s-pool splits across Scalar
    (activation+accum_out) and Vector (tensor_scalar add/max + accum_out);
    descale 1/512^2 is folded into b1 (host: b1 *= 512^2) and psc_scale.
  - W2 applied as 16 accumulating bf16 N=1 matmuls against bf16 psc.
  - software pipeline: each invocation is split into phase A (index calc,
    gathers, PE transposes, PSUM->SBUF casts) and phase B (DoubleRow
    matmuls, relu/pool, W2). Emission order A(0), A(1), B(0), A(2), B(1)...
    keeps the PE queue free of copy-wait stalls (PE queues are strict
    program order), which also keeps HAM at full clock. Group-1 gating
    gathers/MLP are emitted mid-group-0 for the same reason.
    Whole-pipeline numpy sim: rel err ~3.9e-3 (threshold 2e-2).

HW gotchas (verified on device):
  - indirect DMA consumes exactly ONE index per destination partition —
    multi-index-per-partition gathers return garbage.
  - DoubleRow rhs columns must be contiguous (stride-2 fp8 column APs halve
    the stream rate: ~460ns vs ~240ns per MM).
  - Q7/SWDGE descriptor generation is serial: ~1.1us per 128-row indirect
    gather, ~4.8us per 512-token dma_gather ucode call. Budget it.
"""

import os
import sys

for _p in ("/opt/trn_rl_repo", "/root/.axon_site/_ro/trn_rl_repo"):
    if os.path.isdir(_p) and _p not in sys.path:
        sys.path.insert(0, _p)

import numpy as np

import concourse.bacc as bacc
import concourse.tile as tile
import concourse.mybir as mybir
from concourse.bass import IndirectOffsetOnAxis
from concourse.bass_utils import run_bass_kernel_spmd
from concourse.masks import make_identity

F32 = mybir.dt.float32
BF16 = mybir.dt.bfloat16
F8 = mybir.dt.float8e4
I32 = mybir.dt.int32
I16 = mybir.dt.int16
U8 = mybir.dt.uint8
U32 = mybir.dt.uint32

V, D, H, E, C, TOPK = 16000, 1024, 1024, 8, 16, 2
B, S = 64, 512
GATE_H = 256
NCORES = 8
BL = B // NCORES          # samples per core
DT = D // 128             # 8 d-tiles
HT = H // 128             # 8 h-tiles
ST = S // 128             # 4 s-tiles
MT = GATE_H // 128        # 2 gate-hidden tiles
NGRP = 2                  # sample groups per core (pipelining)
GBL = BL // NGRP          # samples per group
NINV = BL * TOPK          # expert invocations per core
GB_BATCH = 1              # gating samples per dma_gather call

FP8_SCALE = 512.0         # exp_emb/exp_w1 host-side scale into e4m3 range

# merged per-expert row: fp8 W1 then a bf16 region (accessed via bitcast)
W1BYTES = DT * H          # 8192 fp8 bytes
# bf16 col offsets within the tail region (in bf16 elements)
W2COL = 0                 # W2 hi  (HT*C = 128 cols)
W2LO = W2COL + HT * C     # 128    W2 lo
B1COL = W2LO + HT * C     # 256    b1 * FP8_SCALE^2 (HT cols)
B2COL = B1COL + HT        # 264    b2 (1 col, partitions 0..C-1)
WRCOLS = 272              # bf16 tail length
WROWB = W1BYTES + 2 * WRCOLS  # total row bytes (8736)

_compiled = {}
last_results = None       # BassKernelResults of the most recent run (for test.py)


def build_program(reps=1):
    """reps>1 repeats the whole compute body (benchmarking aid)."""
    nc = bacc.Bacc(
        "TRN2", target_bir_lowering=False, debug=False, num_devices=NCORES,
        dynamic_dma_scratch_size=32768,
    )
    act = mybir.ActivationFunctionType

    x_t = nc.dram_tensor("x_loc", [BL, S], I32, kind="ExternalInput")
    xg_t = nc.dram_tensor("xg16", [128, NGRP, GBL * S // 16], I16, kind="ExternalInput")
    emb_t = nc.dram_tensor("emb", [V, D], BF16, kind="ExternalInput")
    eemb_t = nc.dram_tensor("eemb", [E * V, D], BF16, kind="ExternalInput")
    wfull_t = nc.dram_tensor("wfull", [E * 128, WROWB], U8, kind="ExternalInput")
    gw1_t = nc.dram_tensor("gw1", [D, GATE_H], F32, kind="ExternalInput")
    gb1_t = nc.dram_tensor("gb1", [128, MT], F32, kind="ExternalInput")
    gw2_t = nc.dram_tensor("gw2", [GATE_H, E], F32, kind="ExternalInput")
    gb2_t = nc.dram_tensor("gb2", [E, 1], F32, kind="ExternalInput")
    out_t = nc.dram_tensor("out", [BL, C], F32, kind="ExternalOutput")
    dbg_t = None
    if os.environ.get("KDBG") == "1":
        dbg_t = nc.dram_tensor("dbg", [NGRP, GBL, 8], F32, kind="ExternalOutput")
        dbg2_t = nc.dram_tensor("dbg2", [NINV, 128, HT], F32, kind="ExternalOutput")
        dbg3_t = nc.dram_tensor("dbg3", [NGRP, 128, DT, GBL], F32, kind="ExternalOutput")

    with tile.TileContext(nc) as tc:
        with (
            tc.tile_pool(name="const", bufs=1) as cpool,
            tc.tile_pool(name="dram", bufs=1, space="DRAM") as dpool,
        ):
            # ---- constants ----
            id_bf = cpool.tile([128, 128], BF16)
            make_identity(nc, id_bf[:, :])
            id_f = cpool.tile([128, 128], F32)
            make_identity(nc, id_f[:, :])
            ones_m = cpool.tile([1, 128], F32)      # lhsT for K=1 broadcast MMs
            nc.vector.memset(ones_m[:, :], 1.0)
            iota_p = cpool.tile([128, 1], I32)      # value = partition index
            nc.gpsimd.iota(iota_p[:, :], pattern=[[0, 1]], base=0, channel_multiplier=1)

            # token ids, transposed: xt[p, b, t] = x[b, t*128+p]
            xt = cpool.tile([128, BL, ST], I32)
            nc.sync.dma_start(
                out=xt[:, :, :], in_=x_t[:, :].rearrange("b (t p) -> p b t", p=128)
            )
            # per-group wrapped int16 gather indices (4 samples concatenated)
            xg = cpool.tile([128, NGRP, GBL * S // 16], I16)
            nc.sync.dma_start(out=xg[:, :, :], in_=xg_t[:, :, :])

            gb1_sb = cpool.tile([128, MT], F32)
            nc.sync.dma_start(out=gb1_sb[:, :], in_=gb1_t[:, :])
            gb2_sb = cpool.tile([E, 1], F32)
            nc.sync.dma_start(out=gb2_sb[:, :], in_=gb2_t[:, :])
            gw1_sb = cpool.tile([128, DT, GATE_H], F32)
            nc.sync.dma_start(
                out=gw1_sb[:, :, :], in_=gw1_t[:, :].rearrange("(j p) g -> p j g", p=128)
            )
            gw2_sb = cpool.tile([128, MT, E], F32)
            nc.sync.dma_start(
                out=gw2_sb[:, :, :], in_=gw2_t[:, :].rearrange("(m p) e -> p m e", p=128)
            )

            consts = dict(
                id_bf=id_bf, id_f=id_f, ones_m=ones_m, iota_p=iota_p, xt=xt, xg=xg,
                gb1_sb=gb1_sb, gb2_sb=gb2_sb, gw1_sb=gw1_sb, gw2_sb=gw2_sb,
            )
            tensors = dict(
                emb_t=emb_t, eemb_t=eemb_t, wfull_t=wfull_t, out_t=out_t,
            )
            if dbg_t is not None:
                tensors.update(dbg_t=dbg_t, dbg2_t=dbg2_t, dbg3_t=dbg3_t)
            # chain tile serializes reps so the benchmark differential is honest
            chain = None
            if reps > 1:
                chain = cpool.tile([1, 1], F32)
                nc.vector.memset(chain[:, :], 0.0)
            for rep in range(reps):
                _body_once(nc, tc, act, rep, dpool, consts, tensors, chain)

    nc.compile()
    return nc


def _body_once(nc, tc, act, rep, dpool, cn, tn, chain=None):
    sfx = f"_r{rep}"
    id_bf, id_f = cn["id_bf"], cn["id_f"]
    ones_m, iota_p = cn["ones_m"], cn["iota_p"]
    xt, xg = cn["xt"], cn["xg"]
    gb1_sb, gb2_sb, gw1_sb, gw2_sb = cn["gb1_sb"], cn["gb2_sb"], cn["gw1_sb"], cn["gw2_sb"]
    emb_t, eemb_t, wfull_t, out_t = (
        tn["emb_t"], tn["eemb_t"], tn["wfull_t"], tn["out_t"],
    )
    dbg_t, dbg2_t, dbg3_t = tn.get("dbg_t"), tn.get("dbg2_t"), tn.get("dbg3_t")
    # p = pacc / (S * SCALE^2)  (descale folded here + into host-scaled b1)
    psc_scale = 1.0 / (FP8_SCALE * FP8_SCALE * S)

    with (
        tc.tile_pool(name=f"persist{sfx}", bufs=1) as ppool,
        tc.tile_pool(name=f"bc{sfx}", bufs=2) as bcpool,
        # gating pools
        tc.tile_pool(name=f"gat{sfx}", bufs=2) as gpool,
        tc.tile_pool(name=f"gsb{sfx}", bufs=2) as gspool,
        tc.tile_pool(name=f"gpss{sfx}", bufs=2, space="PSUM") as gps_s,
        # expert pools
        tc.tile_pool(name=f"exi{sfx}", bufs=3) as xipool,
        tc.tile_pool(name=f"etok{sfx}", bufs=3) as tokpool,
        tc.tile_pool(name=f"ew{sfx}", bufs=3) as wpool,
        tc.tile_pool(name=f"ett{sfx}", bufs=3) as ttpool,
        tc.tile_pool(name=f"esm{sfx}", bufs=3) as smpool,
        tc.tile_pool(name=f"ejunk{sfx}", bufs=2) as junkpool,
        tc.tile_pool(name=f"epst{sfx}", bufs=2, space="PSUM") as eps_t,
        tc.tile_pool(name=f"epsz{sfx}", bufs=2, space="PSUM") as eps_z,
        tc.tile_pool(name=f"epso{sfx}", bufs=1, space="PSUM") as eps_o,
    ):
        out_acc = ppool.tile([C, BL], F32)
        nc.vector.memset(out_acc[:, :], 0.0)
        pts = [None, None]   # per-group pooled^T [128, DT, GBL]
        BC = [None, None]    # per-group broadcast scalars (BCf, BCi)

        def gate_gather(g):
            """Q7 gather + DVE reduce for group g's pooled embeddings."""
            nsub = GBL // GB_BATCH
            p = gspool.tile([128, DT, GBL], F32, tag="pts")
            for sub in range(nsub):
                n = GB_BATCH * S
                gtokT = gpool.tile([128, DT, n], BF16, tag="gtokT")
                nc.gpsimd.dma_gather(
                    out_ap=gtokT[:, :, :],
                    in_ap=emb_t[:, :],
                    idxs_ap=xg[:, g, sub * (n // 16) : (sub + 1) * (n // 16)],
                    num_idxs=n,
                    num_idxs_reg=n,
                    elem_size=D,
                    transpose=True,
                )
                nc.vector.tensor_reduce(
                    out=p[:, :, sub * GB_BATCH : (sub + 1) * GB_BATCH],
                    in_=gtokT[:, :, :].rearrange(
                        "p j (bl s) -> p j bl s", bl=GB_BATCH
                    ),
                    axis=mybir.AxisListType.X,
                    op=mybir.AluOpType.add,
                )
            if dbg3_t is not None:
                nc.sync.dma_start(out=dbg3_t[g], in_=p[:, :, :])
            pts[g] = p

        def gate_mlp(g):
            """Gate MLP + top-2 + per-(b,k) scalar broadcast for group g."""
            p = pts[g]
            hR = gspool.tile([128, MT, GBL], F32, tag="hR")
            for m in range(MT):
                h_ps = gps_s.tile([128, GBL], F32, tag="gmisc")
                for j in range(DT):
                    nc.tensor.matmul(
                        out=h_ps[:, :],
                        lhsT=gw1_sb[:, j, m * 128 : (m + 1) * 128],
                        rhs=p[:, j, :],
                        start=(j == 0),
                        stop=(j == DT - 1),
                    )
                nc.scalar.activation(
                    out=hR[:, m, :], in_=h_ps[:, :], func=act.Relu,
                    bias=gb1_sb[:, m : m + 1],
                )

            l_ps = gps_s.tile([E, GBL], F32, tag="gmisc")
            for m in range(MT):
                nc.tensor.matmul(
                    out=l_ps[:, :],
                    lhsT=gw2_sb[:, m, :],
                    rhs=hR[:, m, :],
                    start=(m == 0),
                    stop=(m == MT - 1),
                )
            l_sb = gspool.tile([E, GBL], F32, tag="l_sb")
            nc.scalar.activation(
                out=l_sb[:, :], in_=l_ps[:, :], func=act.Identity,
                bias=gb2_sb[:, 0:1],
            )
            lt_ps = gps_s.tile([GBL, E], F32, tag="gmisc")
            nc.tensor.matmul(
                out=lt_ps[:, :], lhsT=l_sb[:, :], rhs=id_f[0:E, 0:E],
                start=True, stop=True,
            )
            lt_sb = gspool.tile([GBL, E], F32, tag="lt_sb")
            nc.vector.tensor_copy(lt_sb[:, :], lt_ps[:, :])

            if dbg_t is not None:
                nc.sync.dma_start(out=dbg_t[g, :, 0:E], in_=lt_sb[:, :])
            mx = gspool.tile([GBL, 8], F32, tag="mx")
            mi = gspool.tile([GBL, 8], U32, tag="mi")
            nc.vector.max_with_indices(mx[:, :], mi[:, :], lt_sb[:, :])

            # rw1 = 1/(1+exp(l2-l1)), rw2 = exp(l2-l1)/(1+exp(l2-l1))
            dlt = gspool.tile([GBL, 1], F32, tag="dlt")
            nc.vector.tensor_sub(dlt[:, :], mx[:, 1:2], mx[:, 0:1])
            q = gspool.tile([GBL, 1], F32, tag="q")
            nc.scalar.activation(out=q[:, :], in_=dlt[:, :], func=act.Exp)
            sden = gspool.tile([GBL, 1], F32, tag="sden")
            nc.vector.tensor_scalar_add(sden[:, :], q[:, :], 1.0)
            rw1 = gspool.tile([GBL, 1], F32, tag="rw1")
            nc.vector.reciprocal(rw1[:, :], sden[:, :])
            rw2 = gspool.tile([GBL, 1], F32, tag="rw2")
            nc.vector.tensor_mul(rw2[:, :], q[:, :], rw1[:, :])

            # pack per-(b,k) scalars: cols bl*8 + {0,1}=e*V, {2,3}=e*128,
            # {6,7}=rw ({4,5} unused)
            ei_f = gspool.tile([GBL, TOPK], F32, tag="ei_f")
            nc.vector.tensor_copy(ei_f[:, :], mi[:, 0:TOPK])
            vals = gspool.tile([GBL, 8], F32, tag="vals")
            nc.vector.tensor_scalar_mul(vals[:, 0:2], ei_f[:, :], float(V))
            nc.vector.tensor_scalar_mul(vals[:, 2:4], ei_f[:, :], 128.0)
            nc.vector.tensor_scalar_mul(vals[:, 4:6], ei_f[:, :], 0.0)
            nc.vector.tensor_copy(vals[:, 6:7], rw1[:, :])
            nc.vector.tensor_copy(vals[:, 7:8], rw2[:, :])

            # broadcast across partitions: bounce through DRAM to get a flat
            # [1, GBL*8] row, then K=1 matmul against ones.
            scratch = dpool.tile([GBL, 8], F32, tag=f"scratch{sfx}_{g}")
            nc.sync.dma_start(out=scratch[:, :], in_=vals[:, :])
            if chain is not None and g == 0:
                # unused col 4: forces rep r to wait on rep r-1's result
                nc.sync.dma_start(out=scratch[0:1, 4:5], in_=chain[0:1, 0:1])
            flat = gspool.tile([1, GBL * 8], F32, tag="flat")
            nc.sync.dma_start(
                out=flat[0:1, :].rearrange("p (b c) -> p b c", b=GBL),
                in_=scratch[:, :],
            )
            bc_ps = gps_s.tile([128, GBL * 8], F32, tag="gmisc")
            nc.tensor.matmul(
                out=bc_ps[:, :], lhsT=ones_m[:, :], rhs=flat[0:1, :],
                start=True, stop=True,
            )
            BCf = bcpool.tile([128, GBL * 8], F32, tag="bcf")
            BCi = bcpool.tile([128, GBL * 8], I32, tag="bci")
            nc.vector.tensor_copy(BCf[:, :], bc_ps[:, :])
            nc.vector.tensor_copy(BCi[:, :], bc_ps[:, :])  # cast f32->i32
            BC[g] = (BCf, BCi)

        def phase_a(i):
            """Gathers + PE transpose + fp8 cast for invocation i. Returns ctx."""
            g, r = divmod(i, GBL * TOPK)
            bl, k = divmod(r, TOPK)
            b = g * GBL + bl
            BCf, BCi = BC[g]
            cEV = bl * 8 + k
            cE128 = bl * 8 + 2 + k

            tok_idx = xipool.tile([128, ST], I32, tag="tok_idx")
            nc.vector.tensor_add(
                tok_idx[:, :],
                xt[:, b, :],
                BCi[:, cEV : cEV + 1].to_broadcast([128, ST]),
            )
            w_idx = xipool.tile([128, 1], I32, tag="w_idx")
            nc.vector.tensor_add(
                w_idx[:, :], iota_p[:, :], BCi[:, cE128 : cE128 + 1]
            )

            # bf16 token rows (pre-scaled x512 on host)
            tok = tokpool.tile([128, ST, D], BF16, tag="tok")
            for t in range(ST):
                nc.gpsimd.indirect_dma_start(
                    out=tok[:, t, :],
                    out_offset=None,
                    in_=eemb_t[:, :],
                    in_offset=IndirectOffsetOnAxis(ap=tok_idx[:, t : t + 1], axis=0),
                )
            # merged per-expert weights: W1 fp8 + bf16 tail
            wg = wpool.tile([128, WROWB], U8, tag="wg")
            nc.gpsimd.indirect_dma_start(
                out=wg[:, :],
                out_offset=None,
                in_=wfull_t[:, :],
                in_offset=IndirectOffsetOnAxis(ap=w_idx[:, :], axis=0),
            )
            wr_bf = wg[:, W1BYTES:WROWB].bitcast(BF16)   # [128, WRCOLS] bf16 view
            b1f = smpool.tile([128, HT], F32, tag="b1f")
            nc.vector.tensor_copy(b1f[:, :], wr_bf[:, B1COL : B1COL + HT])
            b2f = smpool.tile([C, 1], F32, tag="b2f")
            nc.vector.tensor_copy(b2f[:, :], wr_bf[0:C, B2COL : B2COL + 1])

            # transpose tok -> tokT[d, s] via matmul against identity;
            # PSUM->SBUF copy casts to fp8 (alternating Vector/Scalar)
            tokT = ttpool.tile([128, DT, S], F8, tag="tokT")
            for j in range(DT):
                tp = eps_t.tile([128, S], F32, tag="tp")
                for t in range(ST):
                    nc.tensor.matmul(
                        out=tp[:, t * 128 : (t + 1) * 128],
                        lhsT=tok[:, t, j * 128 : (j + 1) * 128],
                        rhs=id_bf[:, :],
                        start=True,
                        stop=True,
                    )
                if j % 2 == 0:
                    nc.vector.tensor_copy(tokT[:, j, :], tp[:, :])
                else:
                    nc.scalar.activation(
                        out=tokT[:, j, :], in_=tp[:, :], func=act.Copy
                    )
            return dict(b=b, wg=wg, b1f=b1f, b2f=b2f, tokT=tokT, BCf=BCf,
                        cRW=bl * 8 + 6 + k, i=i)

        def phase_b(ctx):
            """DoubleRow z-matmul + relu/pool + W2 for one invocation."""
            b, wg, b1f, b2f, tokT = ctx["b"], ctx["wg"], ctx["b1f"], ctx["b2f"], ctx["tokT"]
            BCf, cRW = ctx["BCf"], ctx["cRW"]
            wview = wg[:, 0:W1BYTES].bitcast(F8).rearrange("p (a h) -> p a h", a=DT)
            wr_bf = wg[:, W1BYTES:WROWB].bitcast(BF16)

            pacc = smpool.tile([128, HT], F32, tag="pacc")
            for j2 in range(HT):
                z_ps = eps_z.tile([128, S], F32, tag="z")
                for tp2 in range(DT // 2):
                    nc.tensor.matmul(
                        out=z_ps[:, :],
                        lhsT=wview[:, 2 * tp2 : 2 * tp2 + 2, j2 * 128 : (j2 + 1) * 128],
                        rhs=tokT[:, 2 * tp2 : 2 * tp2 + 2, :],
                        start=(tp2 == 0),
                        stop=(tp2 == DT // 2 - 1),
                        perf_mode=mybir.MatmulPerfMode.DoubleRow,
                    )
                zjunk = junkpool.tile([128, S], BF16, tag="zjunk")
                if j2 % 2 == 0:
                    # relu(z + b1*S^2), sum over s into pacc col
                    nc.scalar.activation(
                        out=zjunk[:, :],
                        in_=z_ps[:, :],
                        func=act.Relu,
                        bias=b1f[:, j2 : j2 + 1],
                        accum_out=pacc[:, j2 : j2 + 1],
                    )
                else:
                    # relu on DVE: elementwise (z + b1) max 0, then add-reduce
                    nc.vector.tensor_scalar(
                        out=zjunk[:, :],
                        in0=z_ps[:, :],
                        scalar1=b1f[:, j2 : j2 + 1],
                        scalar2=0.0,
                        op0=mybir.AluOpType.add,
                        op1=mybir.AluOpType.max,
                    )
                    nc.vector.tensor_reduce(
                        out=pacc[:, j2 : j2 + 1],
                        in_=zjunk[:, :],
                        axis=mybir.AxisListType.X,
                        op=mybir.AluOpType.add,
                    )

            if dbg2_t is not None:
                nc.sync.dma_start(out=dbg2_t[ctx["i"]], in_=pacc[:, :])
            # p (bf16) = pacc / (S*SCALE^2); W2 applied hi+lo bf16
            psc = smpool.tile([128, HT], BF16, tag="psc")
            nc.vector.tensor_scalar_mul(psc[:, :], pacc[:, :], psc_scale)

            eo_ps = eps_o.tile([C, 1], F32, tag="eo")
            for j2 in range(HT):
                nc.tensor.matmul(
                    out=eo_ps[:, :],
                    lhsT=wr_bf[:, W2COL + j2 * C : W2COL + (j2 + 1) * C],
                    rhs=psc[:, j2 : j2 + 1],
                    start=(j2 == 0),
                    stop=False,
                )
            for j2 in range(HT):
                nc.tensor.matmul(
                    out=eo_ps[:, :],
                    lhsT=wr_bf[:, W2LO + j2 * C : W2LO + (j2 + 1) * C],
                    rhs=psc[:, j2 : j2 + 1],
                    start=False,
                    stop=(j2 == HT - 1),
                )
            eo1 = smpool.tile([C, 1], F32, tag="eo1")
            nc.scalar.activation(
                out=eo1[:, :], in_=eo_ps[:, :], func=act.Identity,
                bias=b2f[:, 0:1],
            )
            eo2 = smpool.tile([C, 1], F32, tag="eo2")
            nc.vector.tensor_mul(eo2[:, :], eo1[:, :], BCf[0:C, cRW : cRW + 1])
            nc.vector.tensor_add(
                out_acc[:, b : b + 1], out_acc[:, b : b + 1], eo2[:, :]
            )

        # ---- pipelined emission ----
        gate_gather(0)
        gate_mlp(0)
        ctxs = {0: phase_a(0)}
        for i in range(NINV):
            nxt = i + 1
            if nxt < NINV:
                ctxs[nxt] = phase_a(nxt)
            phase_b(ctxs.pop(i))
            # group-1 gating emitted mid-group-0 so its Q7/PE work overlaps
            half = GBL * TOPK
            if i == 1:
                gate_gather(1)
            if i == half - 3:
                gate_mlp(1)

        if chain is not None:
            nc.vector.tensor_copy(chain[0:1, 0:1], out_acc[0:1, 0:1])
        nc.sync.dma_start(
            out=out_t[:, :].rearrange("b c -> c b"), in_=out_acc[:, :]
        )


def _wrap16(idx):
    """Wrap a [N] index vector into the [128, N//16] int16 layout the Q7
    dma_gather ucode expects (16 partitions, replicated 8x)."""
    n = idx.shape[0]
    w = idx.reshape(n // 16, 16).T.astype(np.int16)   # [16, N//16]
    return np.tile(w, (8, 1))                         # [128, N//16]


def _prep_inputs(inputs):
    """Host-side dtype casts + re-layouts shared by all cores."""
    import ml_dtypes

    f32 = np.float32
    bf16 = ml_dtypes.bfloat16
    f8 = ml_dtypes.float8_e4m3

    x = np.asarray(inputs["x"]).astype(np.int32)
    # per-core, per-group wrapped indices (GBL samples concatenated)
    xg16 = np.zeros((NCORES, 128, NGRP, GBL * S // 16), np.int16)
    for c in range(NCORES):
        for g in range(NGRP):
            toks = x[c * BL + g * GBL : c * BL + (g + 1) * GBL].reshape(-1)
            xg16[c, :, g, :] = _wrap16(toks)

    emb = np.asarray(inputs["emb"], dtype=f32).astype(bf16)

    # expert embedding rows: bf16, pre-scaled so the device-side fp8 cast
    # (in the transpose PSUM->SBUF copy) lands in e4m3 range
    exp_emb = (
        np.ascontiguousarray(np.asarray(inputs["exp_emb"], dtype=f32).reshape(E * V, D))
        * FP8_SCALE
    ).astype(bf16)

    # W1: fp8 x512, t-major layout [e*128+p, t*H+h] = W1[e, t*128+p, h],
    # merged with the bf16 tail (W2 hi/lo + b1*S^2 + b2) as raw bytes
    w1 = np.asarray(inputs["exp_w1"], dtype=f32)          # [E, D, H]
    ew1 = w1.reshape(E, DT, 128, H).transpose(0, 2, 1, 3).reshape(E * 128, DT * H)
    wf8 = np.clip(ew1 * FP8_SCALE, -240.0, 240.0).astype(f8)

    w2 = np.asarray(inputs["exp_w2"], dtype=f32)          # [E, H, C]
    ew2 = w2.reshape(E, HT, 128, C).transpose(0, 2, 1, 3).reshape(E * 128, HT * C)
    b1 = np.asarray(inputs["exp_b1"], dtype=f32)          # [E, H]
    b1r = b1.reshape(E, HT, 128).transpose(0, 2, 1).reshape(E * 128, HT)
    b2 = np.asarray(inputs["exp_b2"], dtype=f32)          # [E, C]
    b2slot = np.zeros((E * 128, 1), f32)
    for e in range(E):
        b2slot[e * 128 : e * 128 + C, 0] = b2[e]
    w2hi = ew2.astype(bf16).astype(f32)
    w2lo = ew2 - w2hi
    wallr = np.zeros((E * 128, WRCOLS), f32)
    wallr[:, W2COL : W2COL + HT * C] = w2hi
    wallr[:, W2LO : W2LO + HT * C] = w2lo
    # b1 pre-scaled so relu(z_scaled + b1*S^2) descales via psc_scale
    wallr[:, B1COL : B1COL + HT] = b1r * (FP8_SCALE * FP8_SCALE)
    wallr[:, B2COL : B2COL + 1] = b2slot
    wallr = np.ascontiguousarray(wallr).astype(bf16)

    wfull = np.zeros((E * 128, WROWB), np.uint8)
    wfull[:, :W1BYTES] = wf8.view(np.uint8)
    wfull[:, W1BYTES:] = wallr.view(np.uint8).reshape(E * 128, 2 * WRCOLS)

    # mean over S folded into gate_w1
    gw1 = np.ascontiguousarray(np.asarray(inputs["gate_w1"], dtype=f32) / S)
    gb1 = np.ascontiguousarray(
        np.asarray(inputs["gate_b1"], dtype=f32).reshape(MT, 128).T
    )
    gw2 = np.ascontiguousarray(np.asarray(inputs["gate_w2"], dtype=f32))
    gb2 = np.ascontiguousarray(np.asarray(inputs["gate_b2"], dtype=f32).reshape(E, 1))

    shared = dict(
        emb=emb, eemb=exp_emb, wfull=wfull,
        gw1=gw1, gb1=gb1, gw2=gw2, gb2=gb2,
    )
    return x, xg16, shared


def kernel(**inputs) -> np.ndarray:
    global last_results
    if "nc" not in _compiled:
        _compiled["nc"] = build_program()
    nc = _compiled["nc"]

    x, xg16, shared = _prep_inputs(inputs)
    in_maps = [
        {
            "x_loc": np.ascontiguousarray(x[c * BL : (c + 1) * BL]),
            "xg16": np.ascontiguousarray(xg16[c]),
            **shared,
        }
        for c in range(NCORES)
    ]
    res = run_bass_kernel_spmd(
        nc, in_maps, list(range(NCORES)),
        trace=os.environ.get("KERNEL_TRACE", "0") == "1",
    )
    last_results = res
    out = np.concatenate([res.results[c]["out"] for c in range(NCORES)], axis=0)
    return np.ascontiguousarray(out.astype(np.float32))


# revision 14
# speedup vs baseline: 1.0097x; 1.0097x over previous
"""Trainium2 Bass kernel for an MoE routing module.

Strategy: data-parallel over the batch — each of the 8 NeuronCores runs the
full pipeline (gating -> top-2 -> expert MLPs) for its 8 samples. All
data-dependent expert selection is done with indirect-DMA gathers driven by
index tiles computed on device; there are no collectives and no registers.

v4 (fp8 DoubleRow, software-pipelined):
  - gating: emb table is bf16; ONE dma_gather(transpose=True) per 4-sample
    group (2048 tokens) lands tokens in [d-partition, s] layout; pooling is
    one DVE free-axis reduce into pts[d, j, sample] (mean folded into
    gate_w1 on host). Gate MLP stays fp32 so top-2 matches the fp32
    reference (margin/noise ratio measured 5.4x for this seed).
  - experts: token embeddings gathered bf16 (pre-scaled x512 on host),
    PE-transposed against identity into fp32 PSUM; the PSUM->SBUF copy
    casts to fp8 e4m3 (split across Vector/Scalar engines). W1 is fp8
    (x512) in one merged per-expert row with the bf16 W2hi/W2lo/b1/b2
    region read through a bitcast view (one indirect gather per expert
    use). The big [S,D]@[D,H] matmul runs DoubleRow fp8 (2 k-subtiles per
    pass, contiguous columns). relu+s-pool splits across Scalar
    (activation+accum_out) and Vector (tensor_scalar add/max + accum_out);
    descale 1/512^2 is folded into b1 (host: b1 *= 512^2) and psc_scale.
  - W2 applied as 16 accumulating bf16 N=1 matmuls against bf16 psc.
  - software pipeline: each invocation is split into phase A (index calc,
    gathers, PE transposes, PSUM->SBUF casts) and phase B (DoubleRow
    matmuls, relu/pool, W2). Emission order A(0), A(1), B(0), A(2), B(1)...
    keeps the PE queue free of copy-wait stalls (PE queues are strict
    program order), which also keeps HAM at full clock. Group-1 gating
    gathers/MLP are emitted mid-group-0 for the same reason.
    Whole-pipeline numpy sim: rel err ~3.9e-3 (threshold 2e-2).

HW gotchas (verified on device):
  - indirect DMA consumes exactly ONE index per destination partition —
    multi-index-per-partition gathers return garbage.
  - DoubleRow rhs columns must be contiguous (stride-2 fp8 column APs halve
    the stream rate: ~460ns vs ~240ns per MM).
  - Q7/SWDGE descriptor generation is serial: ~1.1us per 128-row indirect
    gather, ~4.8us per 512-token dma_gather ucode call. Budget it.
"""

import os
import sys

for _p in ("/opt/trn_rl_repo", "/root/.axon_site/_ro/trn_rl_repo"):
    if os.path.isdir(_p) and _p not in sys.path:
        sys.path.insert(0, _p)

import numpy as np

import concourse.bacc as bacc
import concourse.tile as tile
import concourse.mybir as mybir
from concourse.bass import IndirectOffsetOnAxis
from concourse.bass_utils import run_bass_kernel_spmd
from concourse.masks import make_identity

F32 = mybir.dt.float32
BF16 = mybir.dt.bfloat16
F8 = mybir.dt.float8e4
I32 = mybir.dt.int32
I16 = mybir.dt.int16
U8 = mybir.dt.uint8
U32 = mybir.dt.uint32

V, D, H, E, C, TOPK = 16000, 1024, 1024, 8, 16, 2
B, S = 64, 512
GATE_H = 256
NCORES = 8
BL = B // NCORES          # samples per core
DT = D // 128             # 8 d-tiles
HT = H // 128             # 8 h-tiles
ST = S // 128             # 4 s-tiles
MT = GATE_H // 128        # 2 gate-hidden tiles
NGRP = 2                  # sample groups per core (pipelining)
GBL = BL // NGRP          # samples per group
NINV = BL * TOPK          # expert invocations per core
GB_BATCH = 1              # gating samples per dma_gather call

FP8_SCALE = 512.0         # exp_emb/exp_w1 host-side scale into e4m3 range

# merged per-expert row: fp8 W1 then a bf16 region (accessed via bitcast)
W1BYTES = DT * H          # 8192 fp8 bytes
# bf16 col offsets within the tail region (in bf16 elements)
W2COL = 0                 # W2 hi  (HT*C = 128 cols)
W2LO = W2COL + HT * C     # 128    W2 lo
B1COL = W2LO + HT * C     # 256    b1 * FP8_SCALE^2 (HT cols)
B2COL = B1COL + HT        # 264    b2 (1 col, partitions 0..C-1)
WRCOLS = 272              # bf16 tail length
WROWB = W1BYTES + 2 * WRCOLS  # total row bytes (8736)

_compiled = {}
last_results = None       # BassKernelResults of the most recent run (for test.py)


def build_program(reps=1, b1_zero=True):
    """reps>1 repeats the whole compute body (benchmarking aid)."""
    nc = bacc.Bacc(
        "TRN2", target_bir_lowering=False, debug=False, num_devices=NCORES,
        dynamic_dma_scratch_size=32768,
    )
    act = mybir.ActivationFunctionType

    x_t = nc.dram_tensor("x_loc", [BL, S], I32, kind="ExternalInput")
    xg_t = nc.dram_tensor("xg16", [128, NGRP, GBL * S // 16], I16, kind="ExternalInput")
    emb_t = nc.dram_tensor("emb", [V, D], BF16, kind="ExternalInput")
    eemb_t = nc.dram_tensor("eemb", [E * V, D], BF16, kind="ExternalInput")
    wfull_t = nc.dram_tensor("wfull", [E * 128, WROWB], U8, kind="ExternalInput")
    gw1_t = nc.dram_tensor("gw1", [D, GATE_H], F32, kind="ExternalInput")
    gb1_t = nc.dram_tensor("gb1", [128, MT], F32, kind="ExternalInput")
    gw2_t = nc.dram_tensor("gw2", [GATE_H, E], F32, kind="ExternalInput")
    gb2_t = nc.dram_tensor("gb2", [E, 1], F32, kind="ExternalInput")
    out_t = nc.dram_tensor("out", [BL, C], F32, kind="ExternalOutput")
    dbg_t = None
    if os.environ.get("KDBG") == "1":
        dbg_t = nc.dram_tensor("dbg", [NGRP, GBL, 8], F32, kind="ExternalOutput")
        dbg2_t = nc.dram_tensor("dbg2", [NINV, 128, HT], F32, kind="ExternalOutput")
        dbg3_t = nc.dram_tensor("dbg3", [NGRP, 128, DT, GBL], F32, kind="ExternalOutput")

    with tile.TileContext(nc) as tc:
        with (
            tc.tile_pool(name="const", bufs=1) as cpool,
            tc.tile_pool(name="dram", bufs=1, space="DRAM") as dpool,
        ):
            # ---- constants ----
            id_bf = cpool.tile([128, 128], BF16)
            make_identity(nc, id_bf[:, :])
            id_f = cpool.tile([128, 128], F32)
            make_identity(nc, id_f[:, :])
            ones_m = cpool.tile([1, 128], F32)      # lhsT for K=1 broadcast MMs
            nc.vector.memset(ones_m[:, :], 1.0)
            zcol = cpool.tile([128, 1], F32)        # zeros for DVE relu max
            nc.vector.memset(zcol[:, :], 0.0)
            iota_p = cpool.tile([128, 1], I32)      # value = partition index
            nc.gpsimd.iota(iota_p[:, :], pattern=[[0, 1]], base=0, channel_multiplier=1)

            # token ids, transposed: xt[p, b, t] = x[b, t*128+p]
            xt = cpool.tile([128, BL, ST], I32)
            nc.sync.dma_start(
                out=xt[:, :, :], in_=x_t[:, :].rearrange("b (t p) -> p b t", p=128)
            )
            # per-group wrapped int16 gather indices (4 samples concatenated)
            xg = cpool.tile([128, NGRP, GBL * S // 16], I16)
            nc.sync.dma_start(out=xg[:, :, :], in_=xg_t[:, :, :])

            gb1_sb = cpool.tile([128, MT], F32)
            nc.sync.dma_start(out=gb1_sb[:, :], in_=gb1_t[:, :])
            gb2_sb = cpool.tile([E, 1], F32)
            nc.sync.dma_start(out=gb2_sb[:, :], in_=gb2_t[:, :])
            gw1_sb = cpool.tile([128, DT, GATE_H], F32)
            nc.sync.dma_start(
                out=gw1_sb[:, :, :], in_=gw1_t[:, :].rearrange("(j p) g -> p j g", p=128)
            )
            gw2_sb = cpool.tile([128, MT, E], F32)
            nc.sync.dma_start(
                out=gw2_sb[:, :, :], in_=gw2_t[:, :].rearrange("(m p) e -> p m e", p=128)
            )

            consts = dict(
                id_bf=id_bf, id_f=id_f, ones_m=ones_m, iota_p=iota_p, xt=xt, xg=xg, zcol=zcol,
                gb1_sb=gb1_sb, gb2_sb=gb2_sb, gw1_sb=gw1_sb, gw2_sb=gw2_sb,
            )
            tensors = dict(
                emb_t=emb_t, eemb_t=eemb_t, wfull_t=wfull_t, out_t=out_t,
            )
            if dbg_t is not None:
                tensors.update(dbg_t=dbg_t, dbg2_t=dbg2_t, dbg3_t=dbg3_t)
            # chain tile serializes reps so the benchmark differential is honest
            chain = None
            if reps > 1:
                chain = cpool.tile([1, 1], F32)
                nc.vector.memset(chain[:, :], 0.0)
            for rep in range(reps):
                _body_once(nc, tc, act, rep, dpool, consts, tensors, chain,
                           b1_zero=b1_zero)

    nc.compile()
    return nc


def _body_once(nc, tc, act, rep, dpool, cn, tn, chain=None, b1_zero=True):
    sfx = f"_r{rep}"
    id_bf, id_f = cn["id_bf"], cn["id_f"]
    ones_m, iota_p, zcol = cn["ones_m"], cn["iota_p"], cn["zcol"]
    xt, xg = cn["xt"], cn["xg"]
    gb1_sb, gb2_sb, gw1_sb, gw2_sb = cn["gb1_sb"], cn["gb2_sb"], cn["gw1_sb"], cn["gw2_sb"]
    emb_t, eemb_t, wfull_t, out_t = (
        tn["emb_t"], tn["eemb_t"], tn["wfull_t"], tn["out_t"],
    )
    dbg_t, dbg2_t, dbg3_t = tn.get("dbg_t"), tn.get("dbg2_t"), tn.get("dbg3_t")
    # p = pacc / (S * SCALE^2)  (descale folded here + into host-scaled b1)
    psc_scale = 1.0 / (FP8_SCALE * FP8_SCALE * S)

    with (
        tc.tile_pool(name=f"persist{sfx}", bufs=1) as ppool,
        tc.tile_pool(name=f"bc{sfx}", bufs=2) as bcpool,
        # gating pools
        tc.tile_pool(name=f"gat{sfx}", bufs=2) as gpool,
        tc.tile_pool(name=f"gsb{sfx}", bufs=2) as gspool,
        tc.tile_pool(name=f"gpss{sfx}", bufs=2, space="PSUM") as gps_s,
        # expert pools
        tc.tile_pool(name=f"exi{sfx}", bufs=3) as xipool,
        tc.tile_pool(name=f"etok{sfx}", bufs=3) as tokpool,
        tc.tile_pool(name=f"ew{sfx}", bufs=3) as wpool,
        tc.tile_pool(name=f"ett{sfx}", bufs=3) as ttpool,
        tc.tile_pool(name=f"esm{sfx}", bufs=3) as smpool,
        tc.tile_pool(name=f"ejunk{sfx}", bufs=2) as junkpool,
        tc.tile_pool(name=f"epst{sfx}", bufs=2, space="PSUM") as eps_t,
        tc.tile_pool(name=f"epsz{sfx}", bufs=3, space="PSUM") as eps_z,
        tc.tile_pool(name=f"epso{sfx}", bufs=1, space="PSUM") as eps_o,
    ):
        out_acc = ppool.tile([C, BL], F32)
        nc.vector.memset(out_acc[:, :], 0.0)

        # ~5us of dummy matmuls: HAM un-throttles while the first gathers run
        wu_ps = eps_z.tile([128, S], F32, tag="z")
        for _ in range(12):
            nc.tensor.matmul(
                out=wu_ps[:, 0:GATE_H], lhsT=id_f[:, :], rhs=gw1_sb[:, 0, :],
                start=True, stop=True,
            )
        pts = [None, None]   # per-group pooled^T [128, DT, GBL]
        BC = [None, None]    # per-group broadcast scalars (BCf, BCi)

        def gate_gather(g):
            """Q7 gather + DVE reduce for group g's pooled embeddings."""
            nsub = GBL // GB_BATCH
            p = gspool.tile([128, DT, GBL], F32, tag="pts")
            for sub in range(nsub):
                n = GB_BATCH * S
                gtokT = gpool.tile([128, DT, n], BF16, tag="gtokT")
                nc.gpsimd.dma_gather(
                    out_ap=gtokT[:, :, :],
                    in_ap=emb_t[:, :],
                    idxs_ap=xg[:, g, sub * (n // 16) : (sub + 1) * (n // 16)],
                    num_idxs=n,
                    num_idxs_reg=n,
                    elem_size=D,
                    transpose=True,
                )
                nc.vector.tensor_reduce(
                    out=p[:, :, sub * GB_BATCH : (sub + 1) * GB_BATCH],
                    in_=gtokT[:, :, :].rearrange(
                        "p j (bl s) -> p j bl s", bl=GB_BATCH
                    ),
                    axis=mybir.AxisListType.X,
                    op=mybir.AluOpType.add,
                )
            if dbg3_t is not None:
                nc.sync.dma_start(out=dbg3_t[g], in_=p[:, :, :])
            pts[g] = p

        def gate_mlp(g):
            """Gate MLP + top-2 + per-(b,k) scalar broadcast for group g."""
            p = pts[g]
            hR = gspool.tile([128, MT, GBL], F32, tag="hR")
            for m in range(MT):
                h_ps = gps_s.tile([128, GBL], F32, tag="gmisc")
                for j in range(DT):
                    nc.tensor.matmul(
                        out=h_ps[:, :],
                        lhsT=gw1_sb[:, j, m * 128 : (m + 1) * 128],
                        rhs=p[:, j, :],
                        start=(j == 0),
                        stop=(j == DT - 1),
                    )
                nc.scalar.activation(
                    out=hR[:, m, :], in_=h_ps[:, :], func=act.Relu,
                    bias=gb1_sb[:, m : m + 1],
                )

            l_ps = gps_s.tile([E, GBL], F32, tag="gmisc")
            for m in range(MT):
                nc.tensor.matmul(
                    out=l_ps[:, :],
                    lhsT=gw2_sb[:, m, :],
                    rhs=hR[:, m, :],
                    start=(m == 0),
                    stop=(m == MT - 1),
                )
            l_sb = gspool.tile([E, GBL], F32, tag="l_sb")
            nc.scalar.activation(
                out=l_sb[:, :], in_=l_ps[:, :], func=act.Identity,
                bias=gb2_sb[:, 0:1],
            )
            lt_ps = gps_s.tile([GBL, E], F32, tag="gmisc")
            nc.tensor.matmul(
                out=lt_ps[:, :], lhsT=l_sb[:, :], rhs=id_f[0:E, 0:E],
                start=True, stop=True,
            )
            lt_sb = gspool.tile([GBL, E], F32, tag="lt_sb")
            nc.vector.tensor_copy(lt_sb[:, :], lt_ps[:, :])

            if dbg_t is not None:
                nc.sync.dma_start(out=dbg_t[g, :, 0:E], in_=lt_sb[:, :])
            mx = gspool.tile([GBL, 8], F32, tag="mx")
            mi = gspool.tile([GBL, 8], U32, tag="mi")
            nc.vector.max_with_indices(mx[:, :], mi[:, :], lt_sb[:, :])

            # rw1 = 1/(1+exp(l2-l1)), rw2 = exp(l2-l1)/(1+exp(l2-l1))
            dlt = gspool.tile([GBL, 1], F32, tag="dlt")
            nc.vector.tensor_sub(dlt[:, :], mx[:, 1:2], mx[:, 0:1])
            q = gspool.tile([GBL, 1], F32, tag="q")
            nc.scalar.activation(out=q[:, :], in_=dlt[:, :], func=act.Exp)
            sden = gspool.tile([GBL, 1], F32, tag="sden")
            nc.vector.tensor_scalar_add(sden[:, :], q[:, :], 1.0)
            rw1 = gspool.tile([GBL, 1], F32, tag="rw1")
            nc.vector.reciprocal(rw1[:, :], sden[:, :])
            rw2 = gspool.tile([GBL, 1], F32, tag="rw2")
            nc.vector.tensor_mul(rw2[:, :], q[:, :], rw1[:, :])

            # pack per-(b,k) scalars: cols bl*8 + {0,1}=e*V, {2,3}=e*128,
            # {6,7}=rw ({4,5} unused)
            ei_f = gspool.tile([GBL, TOPK], F32, tag="ei_f")
            nc.vector.tensor_copy(ei_f[:, :], mi[:, 0:TOPK])
            vals = gspool.tile([GBL, 8], F32, tag="vals")
            nc.vector.tensor_scalar_mul(vals[:, 0:2], ei_f[:, :], float(V))
            nc.vector.tensor_scalar_mul(vals[:, 2:4], ei_f[:, :], 128.0)
            nc.vector.tensor_scalar_mul(vals[:, 4:6], ei_f[:, :], 0.0)
            nc.vector.tensor_copy(vals[:, 6:7], rw1[:, :])
            nc.vector.tensor_copy(vals[:, 7:8], rw2[:, :])

            # broadcast across partitions: bounce through DRAM to get a flat
            # [1, GBL*8] row, then K=1 matmul against ones.
            scratch = dpool.tile([GBL, 8], F32, tag=f"scratch{sfx}_{g}")
            nc.sync.dma_start(out=scratch[:, :], in_=vals[:, :])
            if chain is not None and g == 0:
                # unused col 4: forces rep r to wait on rep r-1's result
                nc.sync.dma_start(out=scratch[0:1, 4:5], in_=chain[0:1, 0:1])
            flat = gspool.tile([1, GBL * 8], F32, tag="flat")
            nc.sync.dma_start(
                out=flat[0:1, :].rearrange("p (b c) -> p b c", b=GBL),
                in_=scratch[:, :],
            )
            bc_ps = gps_s.tile([128, GBL * 8], F32, tag="gmisc")
            nc.tensor.matmul(
                out=bc_ps[:, :], lhsT=ones_m[:, :], rhs=flat[0:1, :],
                start=True, stop=True,
            )
            BCf = bcpool.tile([128, GBL * 8], F32, tag="bcf")
            BCi = bcpool.tile([128, GBL * 8], I32, tag="bci")
            nc.vector.tensor_copy(BCf[:, :], bc_ps[:, :])
            nc.vector.tensor_copy(BCi[:, :], bc_ps[:, :])  # cast f32->i32
            BC[g] = (BCf, BCi)

        def phase_a(i):
            """Gathers + PE transpose + fp8 cast for invocation i. Returns ctx."""
            g, r = divmod(i, GBL * TOPK)
            bl, k = divmod(r, TOPK)
            b = g * GBL + bl
            BCf, BCi = BC[g]
            cEV = bl * 8 + k
            cE128 = bl * 8 + 2 + k

            tok_idx = xipool.tile([128, ST], I32, tag="tok_idx")
            nc.vector.tensor_add(
                tok_idx[:, :],
                xt[:, b, :],
                BCi[:, cEV : cEV + 1].to_broadcast([128, ST]),
            )
            w_idx = xipool.tile([128, 1], I32, tag="w_idx")
            nc.vector.tensor_add(
                w_idx[:, :], iota_p[:, :], BCi[:, cE128 : cE128 + 1]
            )

            # bf16 token rows (pre-scaled x512 on host)
            tok = tokpool.tile([128, ST, D], BF16, tag="tok")
            for t in range(ST):
                nc.gpsimd.indirect_dma_start(
                    out=tok[:, t, :],
                    out_offset=None,
                    in_=eemb_t[:, :],
                    in_offset=IndirectOffsetOnAxis(ap=tok_idx[:, t : t + 1], axis=0),
                )
            # merged per-expert weights: W1 fp8 + bf16 tail
            wg = wpool.tile([128, WROWB], U8, tag="wg")
            nc.gpsimd.indirect_dma_start(
                out=wg[:, :],
                out_offset=None,
                in_=wfull_t[:, :],
                in_offset=IndirectOffsetOnAxis(ap=w_idx[:, :], axis=0),
            )
            wr_bf = wg[:, W1BYTES:WROWB].bitcast(BF16)   # [128, WRCOLS] bf16 view
            b1f = smpool.tile([128, HT], F32, tag="b1f")
            nc.vector.tensor_copy(b1f[:, :], wr_bf[:, B1COL : B1COL + HT])
            b2f = smpool.tile([C, 1], F32, tag="b2f")
            nc.vector.tensor_copy(b2f[:, :], wr_bf[0:C, B2COL : B2COL + 1])

            # transpose tok -> tokT[d, s] via matmul against identity;
            # PSUM->SBUF copy casts to fp8 (alternating Vector/Scalar)
            tokT = ttpool.tile([128, DT, S], F8, tag="tokT")
            for j in range(DT):
                tp = eps_t.tile([128, S], F32, tag="tp")
                for t in range(ST):
                    nc.tensor.matmul(
                        out=tp[:, t * 128 : (t + 1) * 128],
                        lhsT=tok[:, t, j * 128 : (j + 1) * 128],
                        rhs=id_bf[:, :],
                        start=True,
                        stop=True,
                    )
                if j % 2 == 0:
                    nc.vector.tensor_copy(tokT[:, j, :], tp[:, :])
                else:
                    nc.scalar.activation(
                        out=tokT[:, j, :], in_=tp[:, :], func=act.Copy
                    )
            return dict(b=b, wg=wg, b1f=b1f, b2f=b2f, tokT=tokT, BCf=BCf,
                        cRW=bl * 8 + 6 + k, i=i)

        def phase_b(ctx):
            """DoubleRow z-matmul + relu/pool + W2 for one invocation."""
            b, wg, b1f, b2f, tokT = ctx["b"], ctx["wg"], ctx["b1f"], ctx["b2f"], ctx["tokT"]
            BCf, cRW = ctx["BCf"], ctx["cRW"]
            wview = wg[:, 0:W1BYTES].bitcast(F8).rearrange("p (a h) -> p a h", a=DT)
            wr_bf = wg[:, W1BYTES:WROWB].bitcast(BF16)

            pacc = smpool.tile([128, HT], F32, tag="pacc")
            for j2 in range(HT):
                z_ps = eps_z.tile([128, S], F32, tag="z")
                for tp2 in range(DT // 2):
                    nc.tensor.matmul(
                        out=z_ps[:, :],
                        lhsT=wview[:, 2 * tp2 : 2 * tp2 + 2, j2 * 128 : (j2 + 1) * 128],
                        rhs=tokT[:, 2 * tp2 : 2 * tp2 + 2, :],
                        start=(tp2 == 0),
                        stop=(tp2 == DT // 2 - 1),
                        perf_mode=mybir.MatmulPerfMode.DoubleRow,
                    )
                zjunk = junkpool.tile([128, S], BF16, tag="zjunk")
                if j2 % 2 == 0 or not b1_zero:
                    # relu(z + b1*S^2), sum over s into pacc col
                    nc.scalar.activation(
                        out=zjunk[:, :],
                        in_=z_ps[:, :],
                        func=act.Relu,
                        bias=b1f[:, j2 : j2 + 1],
                        accum_out=pacc[:, j2 : j2 + 1],
                    )
                else:
                    # relu on DVE: elementwise (z + b1) max 0, then add-reduce
                    nc.vector.tensor_scalar(
                        out=zjunk[:, :],
                        in0=z_ps[:, :],
                        scalar1=b1f[:, j2 : j2 + 1],
                        scalar2=0.0,
                        op0=mybir.AluOpType.add,
                        op1=mybir.AluOpType.max,
                    )
                    nc.vector.tensor_reduce(
                        out=pacc[:, j2 : j2 + 1],
                        in_=zjunk[:, :],
                        axis=mybir.AxisListType.X,
                        op=mybir.AluOpType.add,
                    )

            if dbg2_t is not None:
                nc.sync.dma_start(out=dbg2_t[ctx["i"]], in_=pacc[:, :])
            # p (bf16) = pacc / (S*SCALE^2); W2 applied hi+lo bf16
            psc = smpool.tile([128, HT], BF16, tag="psc")
            nc.vector.tensor_scalar_mul(psc[:, :], pacc[:, :], psc_scale)

            eo_ps = eps_o.tile([C, 1], F32, tag="eo")
            for j2 in range(HT):
                nc.tensor.matmul(
                    out=eo_ps[:, :],
                    lhsT=wr_bf[:, W2COL + j2 * C : W2COL + (j2 + 1) * C],
                    rhs=psc[:, j2 : j2 + 1],
                    start=(j2 == 0),
                    stop=False,
                )
            for j2 in range(HT):
                nc.tensor.matmul(
                    out=eo_ps[:, :],
                    lhsT=wr_bf[:, W2LO + j2 * C : W2LO + (j2 + 1) * C],
                    rhs=psc[:, j2 : j2 + 1],
                    start=False,
                    stop=(j2 == HT - 1),
                )
            eo1 = smpool.tile([C, 1], F32, tag="eo1")
            nc.scalar.activation(
                out=eo1[:, :], in_=eo_ps[:, :], func=act.Identity,
                bias=b2f[:, 0:1],
            )
            eo2 = smpool.tile([C, 1], F32, tag="eo2")
            nc.vector.tensor_mul(eo2[:, :], eo1[:, :], BCf[0:C, cRW : cRW + 1])
            nc.vector.tensor_add(
                out_acc[:, b : b + 1], out_acc[:, b : b + 1], eo2[:, :]
            )

        # ---- pipelined emission ----
        gate_gather(0)
        gate_mlp(0)
        ctxs = {0: phase_a(0)}
        for i in range(NINV):
            nxt = i + 1
            if nxt < NINV:
                ctxs[nxt] = phase_a(nxt)
            phase_b(ctxs.pop(i))
            # group-1 gating emitted mid-group-0 so its Q7/PE work overlaps
            half = GBL * TOPK
            if i == 1:
                gate_gather(1)
            if i == half - 3:
                gate_mlp(1)

        if chain is not None:
            nc.vector.tensor_copy(chain[0:1, 0:1], out_acc[0:1, 0:1])
        nc.sync.dma_start(
            out=out_t[:, :].rearrange("b c -> c b"), in_=out_acc[:, :]
        )


def _wrap16(idx):
    """Wrap a [N] index vector into the [128, N//16] int16 layout the Q7
    dma_gather ucode expects (16 partitions, replicated 8x)."""
    n = idx.shape[0]
    w = idx.reshape(n // 16, 16).T.astype(np.int16)   # [16, N//16]
    return np.tile(w, (8, 1))                         # [128, N//16]


def _prep_inputs(inputs):
    """Host-side dtype casts + re-layouts shared by all cores."""
    import ml_dtypes

    f32 = np.float32
    bf16 = ml_dtypes.bfloat16
    f8 = ml_dtypes.float8_e4m3

    x = np.asarray(inputs["x"]).astype(np.int32)
    # per-core, per-group wrapped indices (GBL samples concatenated)
    xg16 = np.zeros((NCORES, 128, NGRP, GBL * S // 16), np.int16)
    for c in range(NCORES):
        for g in range(NGRP):
            toks = x[c * BL + g * GBL : c * BL + (g + 1) * GBL].reshape(-1)
            xg16[c, :, g, :] = _wrap16(toks)

    emb = np.asarray(inputs["emb"], dtype=f32).astype(bf16)

    # expert embedding rows: bf16, pre-scaled so the device-side fp8 cast
    # (in the transpose PSUM->SBUF copy) lands in e4m3 range
    exp_emb = (
        np.ascontiguousarray(np.asarray(inputs["exp_emb"], dtype=f32).reshape(E * V, D))
        * FP8_SCALE
    ).astype(bf16)

    # W1: fp8 x512, t-major layout [e*128+p, t*H+h] = W1[e, t*128+p, h],
    # merged with the bf16 tail (W2 hi/lo + b1*S^2 + b2) as raw bytes
    w1 = np.asarray(inputs["exp_w1"], dtype=f32)          # [E, D, H]
    ew1 = w1.reshape(E, DT, 128, H).transpose(0, 2, 1, 3).reshape(E * 128, DT * H)
    wf8 = np.clip(ew1 * FP8_SCALE, -240.0, 240.0).astype(f8)

    w2 = np.asarray(inputs["exp_w2"], dtype=f32)          # [E, H, C]
    ew2 = w2.reshape(E, HT, 128, C).transpose(0, 2, 1, 3).reshape(E * 128, HT * C)
    b1 = np.asarray(inputs["exp_b1"], dtype=f32)          # [E, H]
    b1r = b1.reshape(E, HT, 128).transpose(0, 2, 1).reshape(E * 128, HT)
    b2 = np.asarray(inputs["exp_b2"], dtype=f32)          # [E, C]
    b2slot = np.zeros((E * 128, 1), f32)
    for e in range(E):
        b2slot[e * 128 : e * 128 + C, 0] = b2[e]
    w2hi = ew2.astype(bf16).astype(f32)
    w2lo = ew2 - w2hi
    wallr = np.zeros((E * 128, WRCOLS), f32)
    wallr[:, W2COL : W2COL + HT * C] = w2hi
    wallr[:, W2LO : W2LO + HT * C] = w2lo
    # b1 pre-scaled so relu(z_scaled + b1*S^2) descales via psc_scale
    wallr[:, B1COL : B1COL + HT] = b1r * (FP8_SCALE * FP8_SCALE)
    wallr[:, B2COL : B2COL + 1] = b2slot
    wallr = np.ascontiguousarray(wallr).astype(bf16)

    wfull = np.zeros((E * 128, WROWB), np.uint8)
    wfull[:, :W1BYTES] = wf8.view(np.uint8)
    wfull[:, W1BYTES:] = wallr.view(np.uint8).reshape(E * 128, 2 * WRCOLS)

    # mean over S folded into gate_w1
    gw1 = np.ascontiguousarray(np.asarray(inputs["gate_w1"], dtype=f32) / S)
    gb1 = np.ascontiguousarray(
        np.asarray(inputs["gate_b1"], dtype=f32).reshape(MT, 128).T
    )
    gw2 = np.ascontiguousarray(np.asarray(inputs["gate_w2"], dtype=f32))
    gb2 = np.ascontiguousarray(np.asarray(inputs["gate_b2"], dtype=f32).reshape(E, 1))

    shared = dict(
        emb=emb, eemb=exp_emb, wfull=wfull,
        gw1=gw1, gb1=gb1, gw2=gw2, gb2=gb2,
    )
    return x, xg16, shared


def kernel(**inputs) -> np.ndarray:
    global last_results
    b1_zero = not np.any(np.asarray(inputs["exp_b1"]))
    key = ("nc", b1_zero)
    if key not in _compiled:
        _compiled[key] = build_program(b1_zero=b1_zero)
    nc = _compiled[key]

    x, xg16, shared = _prep_inputs(inputs)
    in_maps = [
        {
            "x_loc": np.ascontiguousarray(x[c * BL : (c + 1) * BL]),
            "xg16": np.ascontiguousarray(xg16[c]),
            **shared,
        }
        for c in range(NCORES)
    ]
    res = run_bass_kernel_spmd(
        nc, in_maps, list(range(NCORES)),
        trace=os.environ.get("KERNEL_TRACE", "0") == "1",
    )
    last_results = res
    out = np.concatenate([res.results[c]["out"] for c in range(NCORES)], axis=0)
    return np.ascontiguousarray(out.astype(np.float32))


# revision 18
# speedup vs baseline: 1.0858x; 1.0754x over previous
"""Trainium2 Bass kernel for an MoE routing module.

Strategy: data-parallel over the batch — each of the 8 NeuronCores runs the
full pipeline (gating -> top-2 -> expert MLPs) for its 8 samples. All
data-dependent expert selection is done with indirect-DMA gathers driven by
index tiles computed on device; there are no collectives and no registers.

v4 (fp8 DoubleRow, software-pipelined):
  - gating: emb table is bf16; ONE dma_gather(transpose=True) per 4-sample
    group (2048 tokens) lands tokens in [d-partition, s] layout; pooling is
    one DVE free-axis reduce into pts[d, j, sample] (mean folded into
    gate_w1 on host). Gate MLP stays fp32 so top-2 matches the fp32
    reference (margin/noise ratio measured 5.4x for this seed).
  - experts: token embeddings gathered bf16 (pre-scaled x512 on host),
    PE-transposed against identity into fp32 PSUM; the PSUM->SBUF copy
    casts to fp8 e4m3 (split across Vector/Scalar engines). W1 is fp8
    (x512) in one merged per-expert row with the bf16 W2hi/W2lo/b1/b2
    region read through a bitcast view (one indirect gather per expert
    use). The big [S,D]@[D,H] matmul runs DoubleRow fp8 (2 k-subtiles per
    pass, contiguous columns). relu+s-pool splits across Scalar
    (activation+accum_out) and Vector (tensor_scalar add/max + accum_out);
    descale 1/512^2 is folded into b1 (host: b1 *= 512^2) and psc_scale.
  - W2 applied as 16 accumulating bf16 N=1 matmuls against bf16 psc.
  - software pipeline: each invocation is split into phase A (index calc,
    gathers, PE transposes, PSUM->SBUF casts) and phase B (DoubleRow
    matmuls, relu/pool, W2). Emission order A(0), A(1), B(0), A(2), B(1)...
    keeps the PE queue free of copy-wait stalls (PE queues are strict
    program order), which also keeps HAM at full clock. Group-1 gating
    gathers/MLP are emitted mid-group-0 for the same reason.
    Whole-pipeline numpy sim: rel err ~3.9e-3 (threshold 2e-2).

HW gotchas (verified on device):
  - indirect DMA consumes exactly ONE index per destination partition —
    multi-index-per-partition gathers return garbage.
  - DoubleRow rhs columns must be contiguous (stride-2 fp8 column APs halve
    the stream rate: ~460ns vs ~240ns per MM).
  - Q7/SWDGE descriptor generation is serial: ~1.1us per 128-row indirect
    gather, ~4.8us per 512-token dma_gather ucode call. Budget it.
"""

import os
import sys

for _p in ("/opt/trn_rl_repo", "/root/.axon_site/_ro/trn_rl_repo"):
    if os.path.isdir(_p) and _p not in sys.path:
        sys.path.insert(0, _p)

import numpy as np

import concourse.bacc as bacc
import concourse.tile as tile
import concourse.mybir as mybir
from concourse.bass import IndirectOffsetOnAxis
from concourse.bass_utils import run_bass_kernel_spmd
from concourse.masks import make_identity

F32 = mybir.dt.float32
BF16 = mybir.dt.bfloat16
F8 = mybir.dt.float8e4
I32 = mybir.dt.int32
I16 = mybir.dt.int16
U8 = mybir.dt.uint8
U32 = mybir.dt.uint32

V, D, H, E, C, TOPK = 16000, 1024, 1024, 8, 16, 2
B, S = 64, 512
GATE_H = 256
NCORES = 8
BL = B // NCORES          # samples per core
DT = D // 128             # 8 d-tiles
HT = H // 128             # 8 h-tiles
ST = S // 128             # 4 s-tiles
MT = GATE_H // 128        # 2 gate-hidden tiles
NGRP = 2                  # sample groups per core (pipelining)
GBL = BL // NGRP          # samples per group
NINV = BL * TOPK          # expert invocations per core
GB_BATCH = 1              # gating samples per dma_gather call

FP8_SCALE = 512.0         # exp_emb/exp_w1 host-side scale into e4m3 range

# merged per-expert row: fp8 W1 then a bf16 region (accessed via bitcast)
W1BYTES = DT * H          # 8192 fp8 bytes
# bf16 col offsets within the tail region (in bf16 elements)
W2COL = 0                 # W2 hi  (HT*C = 128 cols)
W2LO = W2COL + HT * C     # 128    W2 lo
B1COL = W2LO + HT * C     # 256    b1 * FP8_SCALE^2 (HT cols)
B2COL = B1COL + HT        # 264    b2 (1 col, partitions 0..C-1)
WRCOLS = 272              # bf16 tail length
WROWB = W1BYTES + 2 * WRCOLS  # total row bytes (8736)

_compiled = {}
last_results = None       # BassKernelResults of the most recent run (for test.py)


def build_program(reps=1, b1_zero=True):
    """reps>1 repeats the whole compute body (benchmarking aid)."""
    nc = bacc.Bacc(
        "TRN2", target_bir_lowering=False, debug=False, num_devices=NCORES,
        dynamic_dma_scratch_size=32768,
    )
    act = mybir.ActivationFunctionType

    x_t = nc.dram_tensor("x_loc", [BL, S], I32, kind="ExternalInput")
    xg_t = nc.dram_tensor("xg16", [128, NGRP, GBL * S // 16], I16, kind="ExternalInput")
    emb_t = nc.dram_tensor("emb", [V, D], BF16, kind="ExternalInput")
    eemb_t = nc.dram_tensor("eemb", [E * V, D], BF16, kind="ExternalInput")
    wfull_t = nc.dram_tensor("wfull", [E * 128, WROWB], U8, kind="ExternalInput")
    gw1_t = nc.dram_tensor("gw1", [D, GATE_H], F32, kind="ExternalInput")
    gb1_t = nc.dram_tensor("gb1", [128, MT], F32, kind="ExternalInput")
    gw2_t = nc.dram_tensor("gw2", [GATE_H, E], F32, kind="ExternalInput")
    gb2_t = nc.dram_tensor("gb2", [E, 1], F32, kind="ExternalInput")
    out_t = nc.dram_tensor("out", [BL, C], F32, kind="ExternalOutput")
    dbg_t = None
    if os.environ.get("KDBG") == "1":
        dbg_t = nc.dram_tensor("dbg", [NGRP, GBL, 8], F32, kind="ExternalOutput")
        dbg2_t = nc.dram_tensor("dbg2", [NINV, 128, HT], F32, kind="ExternalOutput")
        dbg3_t = nc.dram_tensor("dbg3", [NGRP, 128, DT, GBL], F32, kind="ExternalOutput")

    with tile.TileContext(nc) as tc:
        with (
            tc.tile_pool(name="const", bufs=1) as cpool,
            tc.tile_pool(name="dram", bufs=1, space="DRAM") as dpool,
        ):
            # ---- constants ----
            id_bf = cpool.tile([128, 128], BF16)
            make_identity(nc, id_bf[:, :])
            id_f = cpool.tile([128, 128], F32)
            make_identity(nc, id_f[:, :])
            ones_m = cpool.tile([1, 128], F32)      # lhsT for K=1 broadcast MMs
            nc.vector.memset(ones_m[:, :], 1.0)
            zcol = cpool.tile([128, 1], F32)        # zeros for DVE relu max
            nc.vector.memset(zcol[:, :], 0.0)
            iota_p = cpool.tile([128, 1], I32)      # value = partition index
            nc.gpsimd.iota(iota_p[:, :], pattern=[[0, 1]], base=0, channel_multiplier=1)

            # token ids, transposed: xt[p, b, t] = x[b, t*128+p]
            xt = cpool.tile([128, BL, ST], I32)
            nc.sync.dma_start(
                out=xt[:, :, :], in_=x_t[:, :].rearrange("b (t p) -> p b t", p=128)
            )
            # per-group wrapped int16 gather indices (4 samples concatenated)
            xg = cpool.tile([128, NGRP, GBL * S // 16], I16)
            nc.sync.dma_start(out=xg[:, :, :], in_=xg_t[:, :, :])

            gb1_sb = cpool.tile([128, MT], F32)
            nc.sync.dma_start(out=gb1_sb[:, :], in_=gb1_t[:, :])
            gb2_sb = cpool.tile([E, 1], F32)
            nc.sync.dma_start(out=gb2_sb[:, :], in_=gb2_t[:, :])
            gw1_sb = cpool.tile([128, DT, GATE_H], F32)
            nc.sync.dma_start(
                out=gw1_sb[:, :, :], in_=gw1_t[:, :].rearrange("(j p) g -> p j g", p=128)
            )
            gw2_sb = cpool.tile([128, MT, E], F32)
            nc.sync.dma_start(
                out=gw2_sb[:, :, :], in_=gw2_t[:, :].rearrange("(m p) e -> p m e", p=128)
            )

            consts = dict(
                id_bf=id_bf, id_f=id_f, ones_m=ones_m, iota_p=iota_p, xt=xt, xg=xg, zcol=zcol,
                gb1_sb=gb1_sb, gb2_sb=gb2_sb, gw1_sb=gw1_sb, gw2_sb=gw2_sb,
            )
            tensors = dict(
                emb_t=emb_t, eemb_t=eemb_t, wfull_t=wfull_t, out_t=out_t,
            )
            if dbg_t is not None:
                tensors.update(dbg_t=dbg_t, dbg2_t=dbg2_t, dbg3_t=dbg3_t)
            # chain tile serializes reps so the benchmark differential is honest
            chain = None
            if reps > 1:
                chain = cpool.tile([1, 1], F32)
                nc.vector.memset(chain[:, :], 0.0)
            for rep in range(reps):
                _body_once(nc, tc, act, rep, dpool, consts, tensors, chain,
                           b1_zero=b1_zero)

    nc.compile()
    return nc


def _body_once(nc, tc, act, rep, dpool, cn, tn, chain=None, b1_zero=True):
    sfx = f"_r{rep}"
    id_bf, id_f = cn["id_bf"], cn["id_f"]
    ones_m, iota_p, zcol = cn["ones_m"], cn["iota_p"], cn["zcol"]
    xt, xg = cn["xt"], cn["xg"]
    gb1_sb, gb2_sb, gw1_sb, gw2_sb = cn["gb1_sb"], cn["gb2_sb"], cn["gw1_sb"], cn["gw2_sb"]
    emb_t, eemb_t, wfull_t, out_t = (
        tn["emb_t"], tn["eemb_t"], tn["wfull_t"], tn["out_t"],
    )
    dbg_t, dbg2_t, dbg3_t = tn.get("dbg_t"), tn.get("dbg2_t"), tn.get("dbg3_t")
    # p = pacc / (S * SCALE^2)  (descale folded here + into host-scaled b1)
    psc_scale = 1.0 / (FP8_SCALE * FP8_SCALE * S)

    with (
        tc.tile_pool(name=f"persist{sfx}", bufs=1) as ppool,
        tc.tile_pool(name=f"bc{sfx}", bufs=2) as bcpool,
        # gating pools
        tc.tile_pool(name=f"gat{sfx}", bufs=2) as gpool,
        tc.tile_pool(name=f"gsb{sfx}", bufs=2) as gspool,
        tc.tile_pool(name=f"gpss{sfx}", bufs=2, space="PSUM") as gps_s,
        # expert pools
        tc.tile_pool(name=f"exi{sfx}", bufs=3) as xipool,
        tc.tile_pool(name=f"etok{sfx}", bufs=3) as tokpool,
        tc.tile_pool(name=f"ew{sfx}", bufs=3) as wpool,
        tc.tile_pool(name=f"ett{sfx}", bufs=3) as ttpool,
        tc.tile_pool(name=f"esm{sfx}", bufs=3) as smpool,
        tc.tile_pool(name=f"ejunk{sfx}", bufs=2) as junkpool,
        tc.tile_pool(name=f"epst{sfx}", bufs=2, space="PSUM") as eps_t,
        tc.tile_pool(name=f"epsz{sfx}", bufs=3, space="PSUM") as eps_z,
        tc.tile_pool(name=f"epso{sfx}", bufs=1, space="PSUM") as eps_o,
    ):
        out_acc = ppool.tile([C, BL], F32)
        nc.vector.memset(out_acc[:, :], 0.0)

        # ~5us of dummy matmuls: HAM un-throttles while the first gathers run
        wu_ps = eps_z.tile([128, S], F32, tag="z")
        for _ in range(30):
            nc.tensor.matmul(
                out=wu_ps[:, 0:GATE_H], lhsT=id_f[:, :], rhs=gw1_sb[:, 0, :],
                start=True, stop=True,
            )
        pts = [None, None]   # per-group pooled^T [128, DT, GBL]
        BC = [None, None]    # per-group broadcast scalars (BCf, BCi)

        def gate_gather(g, subs=None):
            """Q7 gather + DVE/Scalar reduce for group g's pooled embeddings.
            subs selects which samples to emit (pipelined interleave)."""
            nsub = GBL // GB_BATCH
            if subs is None:
                subs = range(nsub)
            if pts[g] is None:
                pts[g] = gspool.tile([128, DT, GBL], F32, tag="pts", name=f"pts_g{g}{sfx}")
            p = pts[g]
            for sub in subs:
                n = GB_BATCH * S
                gtokT = gpool.tile([128, DT, n], BF16, tag="gtokT")
                nc.gpsimd.dma_gather(
                    out_ap=gtokT[:, :, :],
                    in_ap=emb_t[:, :],
                    idxs_ap=xg[:, g, sub * (n // 16) : (sub + 1) * (n // 16)],
                    num_idxs=n,
                    num_idxs_reg=n,
                    elem_size=D,
                    transpose=True,
                )
                assert GB_BATCH == 1
                nc.vector.tensor_reduce(
                    out=p[:, 0:5, sub : sub + 1].rearrange("p j o -> p (j o)"),
                    in_=gtokT[:, 0:5, :],
                    axis=mybir.AxisListType.X,
                    op=mybir.AluOpType.add,
                )
                for j in range(5, DT):
                    gjunk = junkpool.tile([128, S], BF16, tag="zjunk")
                    nc.scalar.activation(
                        out=gjunk[:, :],
                        in_=gtokT[:, j, :],
                        func=act.Copy,
                        accum_out=p[:, j, sub : sub + 1],
                    )
            if dbg3_t is not None and (GBL - 1) in list(subs):
                nc.sync.dma_start(out=dbg3_t[g], in_=p[:, :, :])

        def gate_mlp(g):
            """Gate MLP + top-2 + per-(b,k) scalar broadcast for group g."""
            p = pts[g]
            hR = gspool.tile([128, MT, GBL], F32, tag="hR")
            for m in range(MT):
                h_ps = gps_s.tile([128, GBL], F32, tag="gmisc")
                for j in range(DT):
                    nc.tensor.matmul(
                        out=h_ps[:, :],
                        lhsT=gw1_sb[:, j, m * 128 : (m + 1) * 128],
                        rhs=p[:, j, :],
                        start=(j == 0),
                        stop=(j == DT - 1),
                    )
                nc.scalar.activation(
                    out=hR[:, m, :], in_=h_ps[:, :], func=act.Relu,
                    bias=gb1_sb[:, m : m + 1],
                )

            l_ps = gps_s.tile([E, GBL], F32, tag="gmisc")
            for m in range(MT):
                nc.tensor.matmul(
                    out=l_ps[:, :],
                    lhsT=gw2_sb[:, m, :],
                    rhs=hR[:, m, :],
                    start=(m == 0),
                    stop=(m == MT - 1),
                )
            l_sb = gspool.tile([E, GBL], F32, tag="l_sb")
            nc.scalar.activation(
                out=l_sb[:, :], in_=l_ps[:, :], func=act.Identity,
                bias=gb2_sb[:, 0:1],
            )
            lt_ps = gps_s.tile([GBL, E], F32, tag="gmisc")
            nc.tensor.matmul(
                out=lt_ps[:, :], lhsT=l_sb[:, :], rhs=id_f[0:E, 0:E],
                start=True, stop=True,
            )
            lt_sb = gspool.tile([GBL, E], F32, tag="lt_sb")
            nc.vector.tensor_copy(lt_sb[:, :], lt_ps[:, :])

            if dbg_t is not None:
                nc.sync.dma_start(out=dbg_t[g, :, 0:E], in_=lt_sb[:, :])
            mx = gspool.tile([GBL, 8], F32, tag="mx")
            mi = gspool.tile([GBL, 8], U32, tag="mi")
            nc.vector.max_with_indices(mx[:, :], mi[:, :], lt_sb[:, :])

            # rw1 = 1/(1+exp(l2-l1)), rw2 = exp(l2-l1)/(1+exp(l2-l1))
            dlt = gspool.tile([GBL, 1], F32, tag="dlt")
            nc.vector.tensor_sub(dlt[:, :], mx[:, 1:2], mx[:, 0:1])
            q = gspool.tile([GBL, 1], F32, tag="q")
            nc.scalar.activation(out=q[:, :], in_=dlt[:, :], func=act.Exp)
            sden = gspool.tile([GBL, 1], F32, tag="sden")
            nc.vector.tensor_scalar_add(sden[:, :], q[:, :], 1.0)
            rw1 = gspool.tile([GBL, 1], F32, tag="rw1")
            nc.vector.reciprocal(rw1[:, :], sden[:, :])
            rw2 = gspool.tile([GBL, 1], F32, tag="rw2")
            nc.vector.tensor_mul(rw2[:, :], q[:, :], rw1[:, :])

            # pack per-(b,k) scalars: cols bl*8 + {0,1}=e*V, {2,3}=e*128,
            # {6,7}=rw ({4,5} unused)
            ei_f = gspool.tile([GBL, TOPK], F32, tag="ei_f")
            nc.vector.tensor_copy(ei_f[:, :], mi[:, 0:TOPK])
            vals = gspool.tile([GBL, 8], F32, tag="vals")
            nc.vector.tensor_scalar_mul(vals[:, 0:2], ei_f[:, :], float(V))
            nc.vector.tensor_scalar_mul(vals[:, 2:4], ei_f[:, :], 128.0)
            nc.vector.tensor_scalar_mul(vals[:, 4:6], ei_f[:, :], 0.0)
            nc.vector.tensor_copy(vals[:, 6:7], rw1[:, :])
            nc.vector.tensor_copy(vals[:, 7:8], rw2[:, :])

            # broadcast across partitions: flatten [GBL, 8] -> [1, GBL*8]
            # with GBL selector matmuls (no DRAM bounce), then K=1 matmul
            # against ones.
            if chain is not None:
                scratch = dpool.tile([GBL, 8], F32, tag=f"scratch{sfx}_{g}")
                nc.sync.dma_start(out=scratch[:, :], in_=vals[:, :])
                if g == 0:
                    # unused col 4: forces rep r to wait on rep r-1's result
                    nc.sync.dma_start(out=scratch[0:1, 4:5], in_=chain[0:1, 0:1])
                flat = gspool.tile([1, GBL * 8], F32, tag="flat")
                nc.sync.dma_start(
                    out=flat[0:1, :].rearrange("p (b c) -> p b c", b=GBL),
                    in_=scratch[:, :],
                )
            else:
                fl_ps = gps_s.tile([1, GBL * 8], F32, tag="gmisc")
                for bl in range(GBL):
                    nc.tensor.matmul(
                        out=fl_ps[0:1, bl * 8 : (bl + 1) * 8],
                        lhsT=id_f[0:GBL, bl : bl + 1],
                        rhs=vals[:, :],
                        start=True,
                        stop=True,
                    )
                flat = gspool.tile([1, GBL * 8], F32, tag="flat")
                nc.vector.tensor_copy(flat[0:1, :], fl_ps[0:1, :])
            bc_ps = gps_s.tile([128, GBL * 8], F32, tag="gmisc")
            nc.tensor.matmul(
                out=bc_ps[:, :], lhsT=ones_m[:, :], rhs=flat[0:1, :],
                start=True, stop=True,
            )
            BCf = bcpool.tile([128, GBL * 8], F32, tag="bcf")
            BCi = bcpool.tile([128, GBL * 8], I32, tag="bci")
            nc.vector.tensor_copy(BCf[:, :], bc_ps[:, :])
            nc.vector.tensor_copy(BCi[:, :], bc_ps[:, :])  # cast f32->i32
            BC[g] = (BCf, BCi)

        def phase_a(i):
            """Gathers + PE transpose + fp8 cast for invocation i. Returns ctx."""
            g, r = divmod(i, GBL * TOPK)
            bl, k = divmod(r, TOPK)
            b = g * GBL + bl
            BCf, BCi = BC[g]
            cEV = bl * 8 + k
            cE128 = bl * 8 + 2 + k

            tok_idx = xipool.tile([128, ST], I32, tag="tok_idx")
            nc.vector.tensor_add(
                tok_idx[:, :],
                xt[:, b, :],
                BCi[:, cEV : cEV + 1].to_broadcast([128, ST]),
            )
            w_idx = xipool.tile([128, 1], I32, tag="w_idx")
            nc.vector.tensor_add(
                w_idx[:, :], iota_p[:, :], BCi[:, cE128 : cE128 + 1]
            )

            # bf16 token rows (pre-scaled x512 on host)
            tok = tokpool.tile([128, ST, D], BF16, tag="tok")
            for t in range(ST):
                nc.gpsimd.indirect_dma_start(
                    out=tok[:, t, :],
                    out_offset=None,
                    in_=eemb_t[:, :],
                    in_offset=IndirectOffsetOnAxis(ap=tok_idx[:, t : t + 1], axis=0),
                )
            # merged per-expert weights: W1 fp8 + bf16 tail
            wg = wpool.tile([128, WROWB], U8, tag="wg")
            nc.gpsimd.indirect_dma_start(
                out=wg[:, :],
                out_offset=None,
                in_=wfull_t[:, :],
                in_offset=IndirectOffsetOnAxis(ap=w_idx[:, :], axis=0),
            )
            wr_bf = wg[:, W1BYTES:WROWB].bitcast(BF16)   # [128, WRCOLS] bf16 view
            b1f = smpool.tile([128, HT], F32, tag="b1f")
            nc.vector.tensor_copy(b1f[:, :], wr_bf[:, B1COL : B1COL + HT])
            b2f = smpool.tile([C, 1], F32, tag="b2f")
            nc.vector.tensor_copy(b2f[:, :], wr_bf[0:C, B2COL : B2COL + 1])

            # transpose tok -> tokT[d, s] via matmul against identity;
            # PSUM->SBUF copy casts to fp8 (alternating Vector/Scalar)
            tokT = ttpool.tile([128, DT, S], F8, tag="tokT")
            for j in range(DT):
                tp = eps_t.tile([128, S], F32, tag="tp")
                for t in range(ST):
                    nc.tensor.matmul(
                        out=tp[:, t * 128 : (t + 1) * 128],
                        lhsT=tok[:, t, j * 128 : (j + 1) * 128],
                        rhs=id_bf[:, :],
                        start=True,
                        stop=True,
                    )
                if j % 2 == 0:
                    nc.vector.tensor_copy(tokT[:, j, :], tp[:, :])
                else:
                    nc.scalar.activation(
                        out=tokT[:, j, :], in_=tp[:, :], func=act.Copy
                    )
            return dict(b=b, wg=wg, b1f=b1f, b2f=b2f, tokT=tokT, BCf=BCf,
                        cRW=bl * 8 + 6 + k, i=i)

        def phase_b(ctx):
            """DoubleRow z-matmul + relu/pool + W2 for one invocation."""
            b, wg, b1f, b2f, tokT = ctx["b"], ctx["wg"], ctx["b1f"], ctx["b2f"], ctx["tokT"]
            BCf, cRW = ctx["BCf"], ctx["cRW"]
            wview = wg[:, 0:W1BYTES].bitcast(F8).rearrange("p (a h) -> p a h", a=DT)
            wr_bf = wg[:, W1BYTES:WROWB].bitcast(BF16)

            pacc = smpool.tile([128, HT], F32, tag="pacc")
            for j2 in range(HT):
                z_ps = eps_z.tile([128, S], F32, tag="z")
                for tp2 in range(DT // 2):
                    nc.tensor.matmul(
                        out=z_ps[:, :],
                        lhsT=wview[:, 2 * tp2 : 2 * tp2 + 2, j2 * 128 : (j2 + 1) * 128],
                        rhs=tokT[:, 2 * tp2 : 2 * tp2 + 2, :],
                        start=(tp2 == 0),
                        stop=(tp2 == DT // 2 - 1),
                        perf_mode=mybir.MatmulPerfMode.DoubleRow,
                    )
                zjunk = junkpool.tile([128, S], BF16, tag="zjunk")
                if j2 % 2 == 0 or not b1_zero:
                    # relu(z + b1*S^2), sum over s into pacc col
                    nc.scalar.activation(
                        out=zjunk[:, :],
                        in_=z_ps[:, :],
                        func=act.Relu,
                        bias=b1f[:, j2 : j2 + 1],
                        accum_out=pacc[:, j2 : j2 + 1],
                    )
                else:
                    # relu on DVE: elementwise (z + b1) max 0, then add-reduce
                    nc.vector.tensor_scalar(
                        out=zjunk[:, :],
                        in0=z_ps[:, :],
                        scalar1=b1f[:, j2 : j2 + 1],
                        scalar2=0.0,
                        op0=mybir.AluOpType.add,
                        op1=mybir.AluOpType.max,
                    )
                    nc.vector.tensor_reduce(
                        out=pacc[:, j2 : j2 + 1],
                        in_=zjunk[:, :],
                        axis=mybir.AxisListType.X,
                        op=mybir.AluOpType.add,
                    )

            if dbg2_t is not None:
                nc.sync.dma_start(out=dbg2_t[ctx["i"]], in_=pacc[:, :])
            # p (bf16) = pacc / (S*SCALE^2); W2 applied hi+lo bf16
            psc = smpool.tile([128, HT], BF16, tag="psc")
            nc.vector.tensor_scalar_mul(psc[:, :], pacc[:, :], psc_scale)

            eo_ps = eps_o.tile([C, 1], F32, tag="eo")
            for j2 in range(HT):
                nc.tensor.matmul(
                    out=eo_ps[:, :],
                    lhsT=wr_bf[:, W2COL + j2 * C : W2COL + (j2 + 1) * C],
                    rhs=psc[:, j2 : j2 + 1],
                    start=(j2 == 0),
                    stop=False,
                )
            for j2 in range(HT):
                nc.tensor.matmul(
                    out=eo_ps[:, :],
                    lhsT=wr_bf[:, W2LO + j2 * C : W2LO + (j2 + 1) * C],
                    rhs=psc[:, j2 : j2 + 1],
                    start=False,
                    stop=(j2 == HT - 1),
                )
            eo1 = smpool.tile([C, 1], F32, tag="eo1")
            nc.scalar.activation(
                out=eo1[:, :], in_=eo_ps[:, :], func=act.Identity,
                bias=b2f[:, 0:1],
            )
            eo2 = smpool.tile([C, 1], F32, tag="eo2")
            nc.vector.tensor_mul(eo2[:, :], eo1[:, :], BCf[0:C, cRW : cRW + 1])
            nc.vector.tensor_add(
                out_acc[:, b : b + 1], out_acc[:, b : b + 1], eo2[:, :]
            )

        # ---- pipelined emission ----
        gate_gather(0)
        gate_mlp(0)
        ctxs = {0: phase_a(0)}
        for i in range(NINV):
            nxt = i + 1
            if nxt < NINV:
                ctxs[nxt] = phase_a(nxt)
            phase_b(ctxs.pop(i))
            # group-1 gating interleaved mid-group-0 (one gather per slot
            # so the Q7 FIFO never stalls the expert token gathers)
            half = GBL * TOPK
            if 0 <= i < GBL:
                gate_gather(1, subs=[i])
            if i == half - 3:
                gate_mlp(1)

        if chain is not None:
            nc.vector.tensor_copy(chain[0:1, 0:1], out_acc[0:1, 0:1])
        nc.sync.dma_start(
            out=out_t[:, :].rearrange("b c -> c b"), in_=out_acc[:, :]
        )


def _wrap16(idx):
    """Wrap a [N] index vector into the [128, N//16] int16 layout the Q7
    dma_gather ucode expects (16 partitions, replicated 8x)."""
    n = idx.shape[0]
    w = idx.reshape(n // 16, 16).T.astype(np.int16)   # [16, N//16]
    return np.tile(w, (8, 1))                         # [128, N//16]


def _prep_inputs(inputs):
    """Host-side dtype casts + re-layouts shared by all cores."""
    import ml_dtypes

    f32 = np.float32
    bf16 = ml_dtypes.bfloat16
    f8 = ml_dtypes.float8_e4m3

    x = np.asarray(inputs["x"]).astype(np.int32)
    # per-core, per-group wrapped indices (GBL samples concatenated)
    xg16 = np.zeros((NCORES, 128, NGRP, GBL * S // 16), np.int16)
    for c in range(NCORES):
        for g in range(NGRP):
            toks = x[c * BL + g * GBL : c * BL + (g + 1) * GBL].reshape(-1)
            xg16[c, :, g, :] = _wrap16(toks)

    emb = np.asarray(inputs["emb"], dtype=f32).astype(bf16)

    # expert embedding rows: bf16, pre-scaled so the device-side fp8 cast
    # (in the transpose PSUM->SBUF copy) lands in e4m3 range
    exp_emb = (
        np.ascontiguousarray(np.asarray(inputs["exp_emb"], dtype=f32).reshape(E * V, D))
        * FP8_SCALE
    ).astype(bf16)

    # W1: fp8 x512, t-major layout [e*128+p, t*H+h] = W1[e, t*128+p, h],
    # merged with the bf16 tail (W2 hi/lo + b1*S^2 + b2) as raw bytes
    w1 = np.asarray(inputs["exp_w1"], dtype=f32)          # [E, D, H]
    ew1 = w1.reshape(E, DT, 128, H).transpose(0, 2, 1, 3).reshape(E * 128, DT * H)
    wf8 = np.clip(ew1 * FP8_SCALE, -240.0, 240.0).astype(f8)

    w2 = np.asarray(inputs["exp_w2"], dtype=f32)          # [E, H, C]
    ew2 = w2.reshape(E, HT, 128, C).transpose(0, 2, 1, 3).reshape(E * 128, HT * C)
    b1 = np.asarray(inputs["exp_b1"], dtype=f32)          # [E, H]
    b1r = b1.reshape(E, HT, 128).transpose(0, 2, 1).reshape(E * 128, HT)
    b2 = np.asarray(inputs["exp_b2"], dtype=f32)          # [E, C]
    b2slot = np.zeros((E * 128, 1), f32)
    for e in range(E):
        b2slot[e * 128 : e * 128 + C, 0] = b2[e]
    w2hi = ew2.astype(bf16).astype(f32)
    w2lo = ew2 - w2hi
    wallr = np.zeros((E * 128, WRCOLS), f32)
    wallr[:, W2COL : W2COL + HT * C] = w2hi
    wallr[:, W2LO : W2LO + HT * C] = w2lo
    # b1 pre-scaled so relu(z_scaled + b1*S^2) descales via psc_scale
    wallr[:, B1COL : B1COL + HT] = b1r * (FP8_SCALE * FP8_SCALE)
    wallr[:, B2COL : B2COL + 1] = b2slot
    wallr = np.ascontiguousarray(wallr).astype(bf16)

    wfull = np.zeros((E * 128, WROWB), np.uint8)
    wfull[:, :W1BYTES] = wf8.view(np.uint8)
    wfull[:, W1BYTES:] = wallr.view(np.uint8).reshape(E * 128, 2 * WRCOLS)

    # mean over S folded into gate_w1
    gw1 = np.ascontiguousarray(np.asarray(inputs["gate_w1"], dtype=f32) / S)
    gb1 = np.ascontiguousarray(
        np.asarray(inputs["gate_b1"], dtype=f32).reshape(MT, 128).T
    )
    gw2 = np.ascontiguousarray(np.asarray(inputs["gate_w2"], dtype=f32))
    gb2 = np.ascontiguousarray(np.asarray(inputs["gate_b2"], dtype=f32).reshape(E, 1))

    shared = dict(
        emb=emb, eemb=exp_emb, wfull=wfull,
        gw1=gw1, gb1=gb1, gw2=gw2, gb2=gb2,
    )
    return x, xg16, shared


def kernel(**inputs) -> np.ndarray:
    global last_results
    b1_zero = not np.any(np.asarray(inputs["exp_b1"]))
    key = ("nc", b1_zero)
    if key not in _compiled:
        _compiled[key] = build_program(b1_zero=b1_zero)
    nc = _compiled[key]

    x, xg16, shared = _prep_inputs(inputs)
    in_maps = [
        {
            "x_loc": np.ascontiguousarray(x[c * BL : (c + 1) * BL]),
            "xg16": np.ascontiguousarray(xg16[c]),
            **shared,
        }
        for c in range(NCORES)
    ]
    res = run_bass_kernel_spmd(
        nc, in_maps, list(range(NCORES)),
        trace=os.environ.get("KERNEL_TRACE", "0") == "1",
    )
    last_results = res
    out = np.concatenate([res.results[c]["out"] for c in range(NCORES)], axis=0)
    return np.ascontiguousarray(out.astype(np.float32))
